# revision 12
# baseline (speedup 1.0000x reference)
"""Trainium2 Bass kernel for causal average pooling (downsampling).

Reference op: out[b, i, d] = mean(x[b, :(i+1)*4, d]) over the time axis,
for x of shape (8, 8192, 512) f32 -> out (8, 2048, 512) f32.

Strategy
--------
Data-parallel over batch: one batch per NeuronCore (8 cores), no
cross-core communication.

Per core the math is, for each channel d independently, a prefix sum
over time sampled every SF=4 steps, scaled by 1/(4(i+1)).  We lay the
data out as [channel partitions, time free-dim] (the host pre-transposes
each batch, which is pure layout) and use the hardware prefix scan
`tensor_tensor_scan` on the vector engine:

    state = (data0[t] + state) + data1[t]

Feeding data0 = x[:, 0::2] and data1 = x[:, 1::2] gives the cumulative
sum over PAIRS: cs2[:, j] = sum(x[:, :2j+2]).  Output i of the reference
needs sum(x[:, :4i+4]) = cs2[:, 2i+1]: a strided gather of the odd
columns times a precomputed 1/(4(i+1)) table.

Each 128-channel tile's time axis is split in two halves so compute can
start after 2 MiB instead of 4 MiB.  The halves are scanned
INDEPENDENTLY (initial=0.0 both — chaining via an AP initial measured
~2.3 us slower per scan); the second half's missing carry (= last column
of the first half's scan) is folded into its output op for free using
scalar_tensor_tensor: out = (cs_local + carry[P,1]) * recip.

Pipeline per core (xT [512 chan, 8192 time], 4 channel tiles x 2 halves):
  SP ring:   8 half-tile x loads (2 MiB each), double-buffered tiles
  ACT ring:  recip table load, then 8 half-tile output stores (1 MiB)
  DVE:       per half: scan (2048 steps) + gather*recip (TT / STT)

Written in raw Bass (not Tile): the walrus build in this container
enforces at most ONE semaphore wait per hardware instruction, so all
cross-engine waits are standalone wait_ge ops.  Each load chunk gets its
own semaphore because completions of back-to-back DMAs on one HWDGE ring
are unordered.
"""

import sys

if "/opt/trn_rl_repo" not in sys.path:
    sys.path.insert(0, "/opt/trn_rl_repo")

import numpy as np

import concourse.bass as bass
import concourse.mybir as mybir
from concourse.bass_utils import run_bass_kernel_spmd

P = 128           # SBUF partitions
SF = 4            # pooling factor
B, L, D = 8, 8192, 512
N_CORES = 8
N_HALF = 2        # time-halves per channel tile


def build_bass(d=D, length=L):
    half = length // 2          # scan steps per tile (pairs)
    out_len = length // SF
    n_ct = d // P
    chunk = length // N_HALF          # x elems per load chunk
    cs_chunk = half // N_HALF         # scan steps per chunk
    o_chunk = out_len // N_HALF       # outputs per chunk
    assert d % P == 0 and length % (2 * SF * N_HALF) == 0

    nc = bass.Bass()
    xT = nc.dram_tensor("xT", [d, length], mybir.dt.float32, kind="ExternalInput")
    recip = nc.dram_tensor(
        "recip", [P, out_len], mybir.dt.float32, kind="ExternalInput"
    )
    outT = nc.dram_tensor(
        "outT", [d, out_len], mybir.dt.float32, kind="ExternalOutput"
    )

    n_chunks = n_ct * N_HALF
    # DVE ops per tile: scan_h0, out_h0, scan_h1, out_h1 (s_cmp +1 each).
    SCAN0, OUT0, SCAN1, OUT1 = 1, 2, 3, 4

    with (
        nc.sbuf_tensor([P, length], mybir.dt.float32) as xt0,
        nc.sbuf_tensor([P, length], mybir.dt.float32) as xt1,
        nc.sbuf_tensor([P, half], mybir.dt.float32) as cs0,
        nc.sbuf_tensor([P, half], mybir.dt.float32) as cs1,
        nc.sbuf_tensor([P, out_len], mybir.dt.float32) as rt,
        nc.sbuf_tensor([P, n_ct, out_len], mybir.dt.float32) as ot,
        nc.semaphore("s_rt") as s_rt,
        nc.semaphore("s_x0") as s_x0,
        nc.semaphore("s_x1") as s_x1,
        nc.semaphore("s_x2") as s_x2,
        nc.semaphore("s_x3") as s_x3,
        nc.semaphore("s_x4") as s_x4,
        nc.semaphore("s_x5") as s_x5,
        nc.semaphore("s_x6") as s_x6,
        nc.semaphore("s_x7") as s_x7,
        nc.semaphore("s_cmp") as s_cmp,
        nc.semaphore("s_out") as s_out,
        nc.Block() as block,
    ):
        xts = [xt0, xt1]
        css = [cs0, cs1]
        s_xs = [s_x0, s_x1, s_x2, s_x3, s_x4, s_x5, s_x6, s_x7][:n_chunks]

        @block.sync
        def _(sync):
            # x loads only on the SP HWDGE ring, half a tile at a time.
            # Concurrent DMAs on a ring share bandwidth round-robin, so the
            # first chunk (the critical-path head) is issued ALONE: later
            # chunks are gated on its completion rather than streamed
            # immediately behind it.
            for ct in range(n_ct):
                for h in range(N_HALF):
                    if ct == 0 and h == 1:
                        sync.wait_ge(s_xs[0], 16)
                    elif ct == 1 and h == 0:
                        sync.wait_ge(s_xs[0], 16)
                    elif ct >= 2:
                        # buffer WAR: the scan that read this half of this
                        # buffer two tiles ago must be done.
                        sync.wait_ge(
                            s_cmp, 4 * (ct - 2) + (SCAN0 if h == 0 else SCAN1)
                        )
                    sync.dma_start(
                        out=xts[ct % 2][:, h * chunk:(h + 1) * chunk],
                        in_=xT[ct * P:(ct + 1) * P, h * chunk:(h + 1) * chunk],
                    ).then_inc(s_xs[ct * N_HALF + h], 16)

        @block.vector
        def _(vector):
            vector.wait_ge(s_rt, 16)
            for ct in range(n_ct):
                cs = css[ct % 2][:, :]
                xtile = xts[ct % 2]
                for h in range(N_HALF):
                    vector.wait_ge(s_xs[ct * N_HALF + h], 16)
                    if ct >= 2:
                        # cs WAW vs out op two tiles ago; trivially satisfied
                        # by DVE program order, stated for the race checker.
                        vector.wait_ge(
                            s_cmp, 4 * (ct - 2) + (OUT0 if h == 0 else OUT1)
                        )
                    xv = (
                        xtile[:, h * chunk:(h + 1) * chunk]
                        .rearrange("p (t two) -> p t two", two=2)
                    )
                    nc.vector.tensor_tensor_scan(
                        cs[:, h * cs_chunk:(h + 1) * cs_chunk],
                        xv[:, :, 0],
                        xv[:, :, 1],
                        0.0,
                        mybir.AluOpType.add,
                        mybir.AluOpType.add,
                    ).then_inc(s_cmp, 1)
                    # scan -> out RAW on the same engine; for the checker.
                    vector.wait_ge(
                        s_cmp, 4 * ct + (SCAN0 if h == 0 else SCAN1)
                    )
                    csv = (
                        cs[:, h * cs_chunk:(h + 1) * cs_chunk]
                        .rearrange("p (t two) -> p t two", two=2)
                    )
                    o_ap = ot[:, ct, h * o_chunk:(h + 1) * o_chunk]
                    r_ap = rt[:, h * o_chunk:(h + 1) * o_chunk]
                    if h == 0:
                        nc.vector.tensor_mul(
                            o_ap, csv[:, :, 1], r_ap
                        ).then_inc(s_cmp, 1)
                    else:
                        # Second half lacks the first half's total: fold the
                        # carry (cs[:, cs_chunk-1], per-partition scalar) into
                        # the scale op: out = (cs_local + carry) * recip.
                        nc.vector.scalar_tensor_tensor(
                            o_ap,
                            csv[:, :, 1],
                            cs[:, cs_chunk - 1:cs_chunk],
                            r_ap,
                            mybir.AluOpType.add,
                            mybir.AluOpType.mult,
                        ).then_inc(s_cmp, 1)

        @block.scalar
        def _(scalar):
            # recip table + output stores on the ACT HWDGE ring.
            scalar.dma_start(out=rt[:, :], in_=recip[:, :]).then_inc(s_rt, 16)
            for ct in range(n_ct):
                for h in range(N_HALF):
                    scalar.wait_ge(s_cmp, 4 * ct + (OUT0 if h == 0 else OUT1))
                    scalar.dma_start(
                        out=outT[ct * P:(ct + 1) * P, h * o_chunk:(h + 1) * o_chunk],
                        in_=ot[:, ct, h * o_chunk:(h + 1) * o_chunk],
                    ).then_inc(s_out, 16)
            # Outputs must be in HBM before the kernel exits.
            scalar.wait_ge(s_out, 16 * n_ct * N_HALF)

    return nc


def _recip_table(out_len):
    r = 1.0 / (SF * np.arange(1, out_len + 1, dtype=np.float64))
    return np.broadcast_to(r.astype(np.float32), (P, out_len)).copy()


def kernel(x: np.ndarray) -> np.ndarray:
    b, length, d = x.shape
    out_len = length // SF
    # One batch per core, channels on partitions: host-side transpose is
    # pure layout so every DMA in the kernel is contiguous.
    xT = np.ascontiguousarray(np.swapaxes(np.asarray(x, dtype=np.float32), 1, 2))
    recip = _recip_table(out_len)
    in_maps = [{"xT": xT[i], "recip": recip} for i in range(b)]
    nc = build_bass(d=d, length=length)
    res = run_bass_kernel_spmd(nc, in_maps, core_ids=list(range(b)))
    outT = np.stack([res.results[i]["outT"] for i in range(b)])
    return np.ascontiguousarray(np.swapaxes(outT, 1, 2))


# revision 13
# speedup vs baseline: 1.1256x; 1.1256x over previous
"""Trainium2 Bass kernel for causal average pooling (downsampling).

Reference op: out[b, i, d] = mean(x[b, :(i+1)*4, d]) over the time axis,
for x of shape (8, 8192, 512) f32 -> out (8, 2048, 512) f32.

Strategy
--------
Data-parallel over batch: one batch per NeuronCore (8 cores), no
cross-core communication.

Per core the math is, for each channel d independently, a prefix sum
over time sampled every SF=4 steps, scaled by 1/(4(i+1)).  We lay the
data out as [channel partitions, time free-dim] (the host pre-transposes
each batch, which is pure layout) and use the hardware prefix scan
`tensor_tensor_scan` on the vector engine:

    state = (data0[t] + state) + data1[t]

Feeding data0 = x[:, 0::2] and data1 = x[:, 1::2] gives the cumulative
sum over PAIRS: cs2[:, j] = sum(x[:, :2j+2]).  Output i of the reference
needs sum(x[:, :4i+4]) = cs2[:, 2i+1]: a strided gather of the odd
columns times a precomputed 1/(4(i+1)) table.

Each 128-channel tile's time axis is split in two halves so compute can
start after 2 MiB instead of 4 MiB.  The halves are scanned
INDEPENDENTLY (initial=0.0 both — chaining via an AP initial measured
~2.3 us slower per scan); the second half's missing carry (= last column
of the first half's scan) is folded into its output op for free using
scalar_tensor_tensor: out = (cs_local + carry[P,1]) * recip.

Pipeline per core (xT [512 chan, 8192 time], 4 channel tiles x 2 halves):
  SP ring:   8 half-tile x loads (2 MiB each), double-buffered tiles
  ACT ring:  recip table load, then 8 half-tile output stores (1 MiB)
  DVE:       per half: scan (2048 steps) + gather*recip (TT / STT)

Written in raw Bass (not Tile): the walrus build in this container
enforces at most ONE semaphore wait per hardware instruction, so all
cross-engine waits are standalone wait_ge ops.  Each load chunk gets its
own semaphore because completions of back-to-back DMAs on one HWDGE ring
are unordered.
"""

import sys

if "/opt/trn_rl_repo" not in sys.path:
    sys.path.insert(0, "/opt/trn_rl_repo")

import numpy as np

import concourse.bass as bass
import concourse.mybir as mybir
from concourse.bass_utils import run_bass_kernel_spmd

P = 128           # SBUF partitions
SF = 4            # pooling factor
B, L, D = 8, 8192, 512
N_CORES = 8
N_HALF = 2        # time-halves per channel tile


def build_bass(d=D, length=L):
    half = length // 2          # scan steps per tile (pairs)
    out_len = length // SF
    n_ct = d // P
    chunk = length // N_HALF          # x elems per load chunk
    cs_chunk = half // N_HALF         # scan steps per chunk
    o_chunk = out_len // N_HALF       # outputs per chunk
    assert d % P == 0 and length % (2 * SF * N_HALF) == 0

    nc = bass.Bass()
    xT = nc.dram_tensor("xT", [d, length], mybir.dt.float32, kind="ExternalInput")
    recip = nc.dram_tensor(
        "recip", [P, out_len], mybir.dt.float32, kind="ExternalInput"
    )
    outT = nc.dram_tensor(
        "outT", [d, out_len], mybir.dt.float32, kind="ExternalOutput"
    )

    n_chunks = n_ct * N_HALF
    # DVE ops per tile: scan_h0, out_h0, scan_h1, out_h1 (s_cmp +1 each).
    SCAN0, OUT0, SCAN1, OUT1 = 1, 2, 3, 4

    with (
        nc.sbuf_tensor([P, length], mybir.dt.float32) as xt0,
        nc.sbuf_tensor([P, length], mybir.dt.float32) as xt1,
        nc.sbuf_tensor([P, half], mybir.dt.float32) as cs0,
        nc.sbuf_tensor([P, half], mybir.dt.float32) as cs1,
        nc.sbuf_tensor([P, out_len], mybir.dt.float32) as rt,
        nc.sbuf_tensor([P, n_ct, out_len], mybir.dt.float32) as ot,
        nc.semaphore("s_rt") as s_rt,
        nc.semaphore("s_x0") as s_x0,
        nc.semaphore("s_x1") as s_x1,
        nc.semaphore("s_x2") as s_x2,
        nc.semaphore("s_x3") as s_x3,
        nc.semaphore("s_x4") as s_x4,
        nc.semaphore("s_x5") as s_x5,
        nc.semaphore("s_x6") as s_x6,
        nc.semaphore("s_x7") as s_x7,
        nc.semaphore("s_cmp") as s_cmp,
        nc.semaphore("s_out") as s_out,
        nc.Block() as block,
    ):
        xts = [xt0, xt1]
        css = [cs0, cs1]
        s_xs = [s_x0, s_x1, s_x2, s_x3, s_x4, s_x5, s_x6, s_x7][:n_chunks]

        @block.sync
        def _(sync):
            # x loads only on the SP HWDGE ring, half a tile at a time.
            for ct in range(n_ct):
                for h in range(N_HALF):
                    if ct >= 2:
                        # buffer WAR: the scan that read this half of this
                        # buffer two tiles ago must be done.
                        sync.wait_ge(
                            s_cmp, 4 * (ct - 2) + (SCAN0 if h == 0 else SCAN1)
                        )
                    sync.dma_start(
                        out=xts[ct % 2][:, h * chunk:(h + 1) * chunk],
                        in_=xT[ct * P:(ct + 1) * P, h * chunk:(h + 1) * chunk],
                    ).then_inc(s_xs[ct * N_HALF + h], 16)

        @block.vector
        def _(vector):
            vector.wait_ge(s_rt, 16)
            for ct in range(n_ct):
                cs = css[ct % 2][:, :]
                xtile = xts[ct % 2]
                for h in range(N_HALF):
                    vector.wait_ge(s_xs[ct * N_HALF + h], 16)
                    if ct >= 2:
                        # cs WAW vs out op two tiles ago; trivially satisfied
                        # by DVE program order, stated for the race checker.
                        vector.wait_ge(
                            s_cmp, 4 * (ct - 2) + (OUT0 if h == 0 else OUT1)
                        )
                    xv = (
                        xtile[:, h * chunk:(h + 1) * chunk]
                        .rearrange("p (t two) -> p t two", two=2)
                    )
                    nc.vector.tensor_tensor_scan(
                        cs[:, h * cs_chunk:(h + 1) * cs_chunk],
                        xv[:, :, 0],
                        xv[:, :, 1],
                        0.0,
                        mybir.AluOpType.add,
                        mybir.AluOpType.add,
                    ).then_inc(s_cmp, 1)
                    # scan -> out RAW on the same engine; for the checker.
                    vector.wait_ge(
                        s_cmp, 4 * ct + (SCAN0 if h == 0 else SCAN1)
                    )
                    csv = (
                        cs[:, h * cs_chunk:(h + 1) * cs_chunk]
                        .rearrange("p (t two) -> p t two", two=2)
                    )
                    o_ap = ot[:, ct, h * o_chunk:(h + 1) * o_chunk]
                    r_ap = rt[:, h * o_chunk:(h + 1) * o_chunk]
                    if h == 0:
                        nc.vector.tensor_mul(
                            o_ap, csv[:, :, 1], r_ap
                        ).then_inc(s_cmp, 1)
                    else:
                        # Second half lacks the first half's total: fold the
                        # carry (cs[:, cs_chunk-1], per-partition scalar) into
                        # the scale op: out = (cs_local + carry) * recip.
                        nc.vector.scalar_tensor_tensor(
                            o_ap,
                            csv[:, :, 1],
                            cs[:, cs_chunk - 1:cs_chunk],
                            r_ap,
                            mybir.AluOpType.add,
                            mybir.AluOpType.mult,
                        ).then_inc(s_cmp, 1)

        @block.scalar
        def _(scalar):
            # recip table + output stores on the ACT HWDGE ring.
            scalar.dma_start(out=rt[:, :], in_=recip[:, :]).then_inc(s_rt, 16)
            for ct in range(n_ct):
                for h in range(N_HALF):
                    scalar.wait_ge(s_cmp, 4 * ct + (OUT0 if h == 0 else OUT1))
                    scalar.dma_start(
                        out=outT[ct * P:(ct + 1) * P, h * o_chunk:(h + 1) * o_chunk],
                        in_=ot[:, ct, h * o_chunk:(h + 1) * o_chunk],
                    ).then_inc(s_out, 16)
            # Outputs must be in HBM before the kernel exits.
            scalar.wait_ge(s_out, 16 * n_ct * N_HALF)

    return nc


def _recip_table(out_len):
    r = 1.0 / (SF * np.arange(1, out_len + 1, dtype=np.float64))
    return np.broadcast_to(r.astype(np.float32), (P, out_len)).copy()


def kernel(x: np.ndarray) -> np.ndarray:
    b, length, d = x.shape
    out_len = length // SF
    # One batch per core, channels on partitions: host-side transpose is
    # pure layout so every DMA in the kernel is contiguous.
    xT = np.ascontiguousarray(np.swapaxes(np.asarray(x, dtype=np.float32), 1, 2))
    recip = _recip_table(out_len)
    in_maps = [{"xT": xT[i], "recip": recip} for i in range(b)]
    nc = build_bass(d=d, length=length)
    res = run_bass_kernel_spmd(nc, in_maps, core_ids=list(range(b)))
    outT = np.stack([res.results[i]["outT"] for i in range(b)])
    return np.ascontiguousarray(np.swapaxes(outT, 1, 2))
